# revision 39
# baseline (speedup 1.0000x reference)
"""Trainium2 Bass kernel for nn_MoEDispatcher (noisy top-k MoE routing + dispatch).

Strategy (expert-parallel, per sharding hint):
  - Host computes top-2 routing from logits and "all-to-all" routes tokens:
    for each expert e, gathers the net rows of tokens whose top-2 contains e
    into a capacity-padded buffer (pre-transposed to [D, C] so the device
    needs no on-chip transposes), plus the router rows for those tokens.
  - Core e computes:
      * softmax-of-top-2 gate for each gathered token (from its logits row)
      * y = gate * (x @ W[e])  via bf16 matmuls, f32 accumulate
      * the dense gates output for its 1/8 token shard
      * the noisy-load partial for its 1/8 token shard
  - Host scatter-adds y rows back per token (the return all-to-all + combine),
    concatenates gate shards, sums load partials.

All model math (gates, load, matmul, gate-weighting) runs on device; the host
does routing-index construction, data movement, and layout only.
"""

import os
import numpy as np
import ml_dtypes

N, E, TOPK, D, H = 8192, 8, 2, 1024, 1024
NCORE = 8
NS = N // NCORE          # tokens per shard for gates/load outputs
C = 2304                 # gather capacity per expert (multiple of 128)
TM = C // 128            # token tiles per core
TSH = NS // 128          # shard token groups
NEG = -1.0e30

TRACE = False            # set True (e.g. from test.py) to neuron-profile
LAST_RESULTS = None      # BassKernelResults of the last kernel() call

_compiled = {}


def _build_nc():
    from contextlib import ExitStack
    import concourse.bass as bass
    import concourse.tile as tile
    from concourse import bacc, mybir

    f32 = mybir.dt.float32
    bf16 = mybir.dt.bfloat16
    AX = mybir.AxisListType
    OP = mybir.AluOpType
    AF = mybir.ActivationFunctionType

    nc = bacc.Bacc("TRN2", target_bir_lowering=False, debug=False)

    # router tensors arrive host-packed as [128, F] so every DMA is 128 dense rows
    xgT = nc.dram_tensor("xgt", [D, C], bf16, kind="ExternalInput").ap()
    wq = nc.dram_tensor("wq", [D, H], bf16, kind="ExternalInput").ap()
    xl = nc.dram_tensor("xl", [128, TM * E], f32, kind="ExternalInput").ap()
    xle = nc.dram_tensor("xle", [128, TM], f32, kind="ExternalInput").ap()
    lg = nc.dram_tensor("lg", [128, TSH * E], f32, kind="ExternalInput").ap()
    cl = nc.dram_tensor("cl", [128, TSH * E], f32, kind="ExternalInput").ap()
    nsd = nc.dram_tensor("nsd", [128, TSH * E], f32, kind="ExternalInput").ap()

    y = nc.dram_tensor("y", [C, H], bf16, kind="ExternalOutput").ap()
    gsh = nc.dram_tensor("gsh", [128, TSH * E], f32, kind="ExternalOutput").ap()
    lsh = nc.dram_tensor("lsh", [1, E], f32, kind="ExternalOutput").ap()

    with tile.TileContext(nc) as tc, ExitStack() as ctx:
        wpool = ctx.enter_context(tc.tile_pool(name="w", bufs=1))
        xpool = ctx.enter_context(tc.tile_pool(name="x", bufs=1))
        rpool = ctx.enter_context(tc.tile_pool(name="r", bufs=1))
        ypool = ctx.enter_context(tc.tile_pool(name="y", bufs=4))
        psum = ctx.enter_context(tc.tile_pool(name="ps", bufs=6, space="PSUM"))
        pl = ctx.enter_context(tc.tile_pool(name="pl", bufs=1, space="PSUM"))

        # ---- stream in W[e] and gathered-token xT tiles (bf16) ----
        # big DMA rows (DMA is ~42ns/descriptor-run); small first chunk and
        # first-needed-first emission so the first matmuls start early
        XCH = [512, 1792]              # column-chunk widths, sum = C
        XOFF = [0, 512]
        M2C = []                       # m-tile -> (chunk, offset-within-chunk)
        for m in range(TM):
            col = m * 128
            c = max(i for i in range(len(XOFF)) if XOFF[i] <= col)
            M2C.append((c, col - XOFF[c]))
        wk = [None] * 8
        xk = [[None] * len(XCH) for _ in range(8)]

        def load_w(k, eng):
            wt = wpool.tile([128, H], bf16, tag=f"w{k}")
            eng.dma_start(wt[:], wq[k * 128:(k + 1) * 128, :])
            wk[k] = wt

        def load_x(c, k, eng):
            c0 = XOFF[c]
            xt = xpool.tile([128, XCH[c]], bf16, tag=f"x{k}c{c}")
            eng.dma_start(xt[:], xgT[k * 128:(k + 1) * 128, c0:c0 + XCH[c]])
            xk[k][c] = xt

        # ---- gate for each gathered token: g = exp(le-m1) / (1+exp(m2-m1)) ----
        xlt = rpool.tile([128, TM * E], f32, tag="xlt")
        nc.gpsimd.dma_start(xlt[:], xl[:])
        xlet = rpool.tile([128, TM], f32, tag="xlet")
        nc.gpsimd.dma_start(xlet[:], xle[:])
        xlt3 = xlt[:].rearrange("p (t e) -> p t e", e=E)

        gm1 = rpool.tile([128, TM], f32, tag="gm1")
        nc.vector.reduce_max(gm1[:], xlt3, axis=AX.X)
        gtmp = rpool.tile([128, TM * E], f32, tag="gtmp")
        nc.vector.tensor_tensor(
            gtmp[:].rearrange("p (t e) -> p t e", e=E),
            xlt3, gm1[:].broadcast_to([128, TM, E]), op=OP.is_equal)
        nc.vector.tensor_scalar(gtmp[:], gtmp[:], NEG, None, op0=OP.mult)
        nc.vector.tensor_add(gtmp[:], gtmp[:], xlt[:])
        gm2 = rpool.tile([128, TM], f32, tag="gm2")
        nc.vector.reduce_max(gm2[:], gtmp[:].rearrange("p (t e) -> p t e", e=E), axis=AX.X)

        gden = rpool.tile([128, TM], f32, tag="gden")
        nc.vector.tensor_sub(gden[:], gm2[:], gm1[:])
        nc.scalar.activation(gden[:], gden[:], AF.Exp)
        nc.vector.tensor_scalar(gden[:], gden[:], 1.0, None, op0=OP.add)
        nc.vector.reciprocal(gden[:], gden[:])
        gg = rpool.tile([128, TM], f32, tag="gg")
        nc.vector.tensor_sub(gg[:], xlet[:], gm1[:])
        nc.scalar.activation(gg[:], gg[:], AF.Exp)
        nc.vector.tensor_mul(gg[:], gg[:], gden[:])


        # alternate issue engines: SP and ACT both issue HWDGE DMAs, halving
        # the per-instruction issue serialization on the critical startup path
        warm = wpool.tile([128, 512], bf16, tag="warm")
        nc.vector.memset(warm[:], 0.0)
        wacc = pl.tile([128, 512], f32, tag="wacc")
        for i in range(20):
            nc.tensor.matmul(wacc[:], warm[:, 0:128], warm[:],
                             start=(i == 0), stop=(i == 19))

        for k in range(8):
            load_w(k, nc.sync)
            load_x(0, k, nc.sync)
        for c in range(1, len(XCH)):
            for k in range(8):
                load_x(c, k, nc.sync)

        # ---- expert compute: y[t, :] = g[t] * (x[t, :] @ W) ----
        for m in range(TM):
            ys = ypool.tile([128, H], bf16, tag="ys")
            for n in range(2):
                acc = psum.tile([128, 512], f32, tag="acc")
                c, co = M2C[m]
                for k in range(8):
                    nc.tensor.matmul(
                        acc[:],
                        xk[k][c][:, co:co + 128],
                        wk[k][:, n * 512:(n + 1) * 512],
                        start=(k == 0), stop=(k == 7),
                    )
                nc.scalar.activation(
                    ys[:, n * 512:(n + 1) * 512], acc[:], AF.Copy,
                    scale=gg[:, m:m + 1])
            nc.scalar.dma_start(y[m * 128:(m + 1) * 128, :], ys[:])

        # ---- gates output for this core's token shard ----
        lgt = rpool.tile([128, TSH * E], f32, tag="lgt")
        nc.gpsimd.dma_start(lgt[:], lg[:])
        lgt3 = lgt[:].rearrange("p (t e) -> p t e", e=E)

        m1 = rpool.tile([128, TSH], f32, tag="m1")
        nc.vector.reduce_max(m1[:], lgt3, axis=AX.X)
        eq1 = rpool.tile([128, TSH * E], f32, tag="eq1")
        nc.vector.tensor_tensor(
            eq1[:].rearrange("p (t e) -> p t e", e=E),
            lgt3, m1[:].broadcast_to([128, TSH, E]), op=OP.is_equal)
        nc.vector.tensor_scalar(eq1[:], eq1[:], NEG, None, op0=OP.mult)
        nc.vector.tensor_add(eq1[:], eq1[:], lgt[:])  # eq1 now = lg with top-1 masked
        m2 = rpool.tile([128, TSH], f32, tag="m2")
        nc.vector.reduce_max(m2[:], eq1[:].rearrange("p (t e) -> p t e", e=E), axis=AX.X)
        eq2 = rpool.tile([128, TSH * E], f32, tag="eq2")
        nc.vector.tensor_tensor(
            eq2[:].rearrange("p (t e) -> p t e", e=E),
            eq1[:].rearrange("p (t e) -> p t e", e=E),
            m2[:].broadcast_to([128, TSH, E]), op=OP.is_equal)
        nc.vector.tensor_scalar(eq2[:], eq2[:], NEG, None, op0=OP.mult)
        nc.vector.tensor_add(eq2[:], eq2[:], eq1[:])  # = lg with top-2 masked
        m3 = rpool.tile([128, TSH], f32, tag="m3")
        nc.vector.reduce_max(m3[:], eq2[:].rearrange("p (t e) -> p t e", e=E), axis=AX.X)

        # gates = (lg >= m2) * exp(lg - m1) / (1 + exp(m2 - m1))
        den = rpool.tile([128, TSH], f32, tag="den")
        nc.vector.tensor_sub(den[:], m2[:], m1[:])
        nc.scalar.activation(den[:], den[:], AF.Exp)
        nc.vector.tensor_scalar(den[:], den[:], 1.0, None, op0=OP.add)
        nc.vector.reciprocal(den[:], den[:])

        gates = rpool.tile([128, TSH * E], f32, tag="gates")
        nc.vector.tensor_tensor(
            gates[:].rearrange("p (t e) -> p t e", e=E),
            lgt3, m1[:].broadcast_to([128, TSH, E]), op=OP.subtract)
        nc.scalar.activation(gates[:], gates[:], AF.Exp)
        msk = rpool.tile([128, TSH * E], f32, tag="msk")
        nc.vector.tensor_tensor(
            msk[:].rearrange("p (t e) -> p t e", e=E),
            lgt3, m2[:].broadcast_to([128, TSH, E]), op=OP.is_ge)
        nc.vector.tensor_mul(gates[:], gates[:], msk[:])
        nc.vector.tensor_tensor(
            gates[:].rearrange("p (t e) -> p t e", e=E),
            gates[:].rearrange("p (t e) -> p t e", e=E),
            den[:].broadcast_to([128, TSH, E]), op=OP.mult)
        nc.scalar.dma_start(gsh[:], gates[:])

        # ---- load partial: sum_t where(lg > m3, cdf((cl-m3)/ns), cdf((cl-m2)/ns)) ----
        clt = rpool.tile([128, TSH * E], f32, tag="clt")
        nc.gpsimd.dma_start(clt[:], cl[:])
        nst = rpool.tile([128, TSH * E], f32, tag="nst")
        nc.gpsimd.dma_start(nst[:], nsd[:])
        clt3 = clt[:].rearrange("p (t e) -> p t e", e=E)
        nc.vector.reciprocal(nst[:], nst[:])

        def norm_cdf(out_t, thr):
            # out = 0.5 * erf((cl - thr)/ns / sqrt(2)) + 0.5
            nc.vector.tensor_tensor(
                out_t[:].rearrange("p (t e) -> p t e", e=E),
                clt3, thr[:].broadcast_to([128, TSH, E]), op=OP.subtract)
            nc.vector.tensor_tensor(out_t[:], out_t[:], nst[:], op=OP.mult)
            # clamp to the Erf LUT's safe range; erf(+-5.66) == 1.0 in f32
            nc.vector.tensor_scalar(out_t[:], out_t[:], 8.0, -8.0,
                                    op0=OP.min, op1=OP.max)
            nc.scalar.activation(out_t[:], out_t[:], AF.Erf,
                                 scale=float(1.0 / np.sqrt(2.0)))
            nc.vector.tensor_scalar(out_t[:], out_t[:], 0.5, 0.5,
                                    op0=OP.mult, op1=OP.add)

        pin = rpool.tile([128, TSH * E], f32, tag="pin")
        norm_cdf(pin, m3)
        pout = rpool.tile([128, TSH * E], f32, tag="pout")
        norm_cdf(pout, m2)
        sel = rpool.tile([128, TSH * E], f32, tag="sel")
        nc.vector.tensor_tensor(
            sel[:].rearrange("p (t e) -> p t e", e=E),
            lgt3, m3[:].broadcast_to([128, TSH, E]), op=OP.is_gt)
        nc.vector.tensor_sub(pin[:], pin[:], pout[:])
        nc.vector.tensor_mul(pin[:], pin[:], sel[:])
        nc.vector.tensor_add(pin[:], pin[:], pout[:])

        lpart = rpool.tile([128, E], f32, tag="lpart")
        nc.vector.reduce_sum(lpart[:], pin[:].rearrange("p (t e) -> p e t", e=E),
                             axis=AX.X)
        ones = rpool.tile([128, 1], f32, tag="ones")
        nc.vector.memset(ones[:], 1.0)
        lacc = pl.tile([1, E], f32, tag="lacc")
        nc.tensor.matmul(lacc[:], ones[:], lpart[:], start=True, stop=True)
        lout = rpool.tile([1, E], f32, tag="lout")
        nc.vector.tensor_copy(lout[:], lacc[:])
        nc.scalar.dma_start(lsh[:], lout[:])

    nc.compile()
    return nc


def _get_nc():
    if "nc" not in _compiled:
        _compiled["nc"] = _build_nc()
    return _compiled["nc"]


def _route(logits):
    """Host-side top-2 routing -> per-expert token index lists (the all-to-all)."""
    order = np.argsort(-logits, axis=1, kind="stable")
    top2 = order[:, :TOPK]
    sels, cnts = [], []
    for e in range(E):
        sel = np.nonzero((top2[:, 0] == e) | (top2[:, 1] == e))[0]
        assert len(sel) <= C, f"expert {e} overflow: {len(sel)} > {C}"
        sels.append(sel)
        cnts.append(len(sel))
    return sels, cnts


def _make_in_maps(net, logits, clean_logits, noise_std, W, sels, cnts):
    def pack(a, t):
        # [t*128, F] -> [128, t*F] matching the on-chip "p (t f)" layout
        return np.ascontiguousarray(
            a.reshape(t, 128, -1).transpose(1, 0, 2).reshape(128, -1))

    in_maps = []
    for e in range(E):
        sel = sels[e]
        idx = np.zeros(C, dtype=np.int64)
        idx[:len(sel)] = sel
        xg = net[idx]
        xg[len(sel):] = 0.0
        xgT = np.ascontiguousarray(xg.T).astype(ml_dtypes.bfloat16)
        xlrows = logits[idx].copy()
        xlrows[len(sel):] = 0.0
        s = slice(e * NS, (e + 1) * NS)
        in_maps.append({
            "xgt": xgT,
            "wq": W[e].astype(ml_dtypes.bfloat16),
            "xl": pack(xlrows, TM),
            "xle": pack(xlrows[:, e:e + 1], TM),
            "lg": pack(logits[s], TSH),
            "cl": pack(clean_logits[s], TSH),
            "nsd": pack(noise_std[s], TSH),
        })
    return in_maps


def kernel(net, logits, clean_logits, noise_std, W):
    global LAST_RESULTS
    from concourse.bass_utils import run_bass_kernel_spmd

    net = np.asarray(net, dtype=np.float32)
    logits = np.asarray(logits, dtype=np.float32)
    clean_logits = np.asarray(clean_logits, dtype=np.float32)
    noise_std = np.asarray(noise_std, dtype=np.float32)
    W = np.asarray(W, dtype=np.float32)

    sels, cnts = _route(logits)
    in_maps = _make_in_maps(net, logits, clean_logits, noise_std, W, sels, cnts)

    nc = _get_nc()
    res = run_bass_kernel_spmd(
        nc, in_maps, core_ids=list(range(NCORE)), trace=TRACE)
    LAST_RESULTS = res

    out = np.zeros((N, H), dtype=np.float32)
    gates = np.empty((N, E), dtype=np.float32)
    load = np.zeros(E, dtype=np.float32)
    for e in range(E):
        r = res.results[e]
        out[sels[e]] += r["y"][:cnts[e]].astype(np.float32)
        gates[e * NS:(e + 1) * NS] = (
            r["gsh"].reshape(128, TSH, E).transpose(1, 0, 2).reshape(NS, E))
        load += r["lsh"].reshape(E)
    return out, gates, load


# revision 40
# speedup vs baseline: 1.0562x; 1.0562x over previous
"""Trainium2 Bass kernel for nn_MoEDispatcher (noisy top-k MoE routing + dispatch).

Strategy (expert-parallel, per sharding hint):
  - Host computes top-2 routing from logits and "all-to-all" routes tokens:
    for each expert e, gathers the net rows of tokens whose top-2 contains e
    into a capacity-padded buffer (pre-transposed to [D, C] so the device
    needs no on-chip transposes), plus the router rows for those tokens.
  - Core e computes:
      * softmax-of-top-2 gate for each gathered token (from its logits row)
      * y = gate * (x @ W[e])  via bf16 matmuls, f32 accumulate
      * the dense gates output for its 1/8 token shard
      * the noisy-load partial for its 1/8 token shard
  - Host scatter-adds y rows back per token (the return all-to-all + combine),
    concatenates gate shards, sums load partials.

All model math (gates, load, matmul, gate-weighting) runs on device; the host
does routing-index construction, data movement, and layout only.
"""

import os
import numpy as np
import ml_dtypes

N, E, TOPK, D, H = 8192, 8, 2, 1024, 1024
NCORE = 8
NS = N // NCORE          # tokens per shard for gates/load outputs
C = 2304                 # gather capacity per expert (multiple of 128)
TM = C // 128            # token tiles per core
TSH = NS // 128          # shard token groups
NEG = -1.0e30

TRACE = False            # set True (e.g. from test.py) to neuron-profile
LAST_RESULTS = None      # BassKernelResults of the last kernel() call

_compiled = {}


def _build_nc():
    from contextlib import ExitStack
    import concourse.bass as bass
    import concourse.tile as tile
    from concourse import bacc, mybir

    f32 = mybir.dt.float32
    bf16 = mybir.dt.bfloat16
    AX = mybir.AxisListType
    OP = mybir.AluOpType
    AF = mybir.ActivationFunctionType

    nc = bacc.Bacc("TRN2", target_bir_lowering=False, debug=False)

    # router tensors arrive host-packed as [128, F] so every DMA is 128 dense rows
    xgT = nc.dram_tensor("xgt", [D, C], bf16, kind="ExternalInput").ap()
    wq = nc.dram_tensor("wq", [D, H], bf16, kind="ExternalInput").ap()
    xl = nc.dram_tensor("xl", [128, TM * E], f32, kind="ExternalInput").ap()
    xle = nc.dram_tensor("xle", [128, TM], f32, kind="ExternalInput").ap()
    lg = nc.dram_tensor("lg", [128, TSH * E], f32, kind="ExternalInput").ap()
    cl = nc.dram_tensor("cl", [128, TSH * E], f32, kind="ExternalInput").ap()
    nsd = nc.dram_tensor("nsd", [128, TSH * E], f32, kind="ExternalInput").ap()

    y = nc.dram_tensor("y", [C, H], bf16, kind="ExternalOutput").ap()
    gsh = nc.dram_tensor("gsh", [128, TSH * E], f32, kind="ExternalOutput").ap()
    lsh = nc.dram_tensor("lsh", [1, E], f32, kind="ExternalOutput").ap()

    with tile.TileContext(nc) as tc, ExitStack() as ctx:
        wpool = ctx.enter_context(tc.tile_pool(name="w", bufs=1))
        xpool = ctx.enter_context(tc.tile_pool(name="x", bufs=1))
        rpool = ctx.enter_context(tc.tile_pool(name="r", bufs=1))
        ypool = ctx.enter_context(tc.tile_pool(name="y", bufs=4))
        psum = ctx.enter_context(tc.tile_pool(name="ps", bufs=6, space="PSUM"))
        pl = ctx.enter_context(tc.tile_pool(name="pl", bufs=1, space="PSUM"))

        # ---- stream in W[e] and gathered-token xT tiles (bf16) ----
        # big DMA rows (DMA is ~42ns/descriptor-run); small first chunk and
        # first-needed-first emission so the first matmuls start early
        XCH = [512, 1792]              # column-chunk widths, sum = C
        XOFF = [0, 512]
        M2C = []                       # m-tile -> (chunk, offset-within-chunk)
        for m in range(TM):
            col = m * 128
            c = max(i for i in range(len(XOFF)) if XOFF[i] <= col)
            M2C.append((c, col - XOFF[c]))
        wk = [None] * 8
        xk = [[None] * len(XCH) for _ in range(8)]

        def load_w(k, eng):
            wt = wpool.tile([128, H], bf16, tag=f"w{k}")
            eng.dma_start(wt[:], wq[k * 128:(k + 1) * 128, :])
            wk[k] = wt

        def load_x(c, k, eng):
            c0 = XOFF[c]
            xt = xpool.tile([128, XCH[c]], bf16, tag=f"x{k}c{c}")
            eng.dma_start(xt[:], xgT[k * 128:(k + 1) * 128, c0:c0 + XCH[c]])
            xk[k][c] = xt

        # ---- gate for each gathered token: g = exp(le-m1) / (1+exp(m2-m1)) ----
        xlt = rpool.tile([128, TM * E], f32, tag="xlt")
        nc.gpsimd.dma_start(xlt[:], xl[:])
        xlet = rpool.tile([128, TM], f32, tag="xlet")
        nc.gpsimd.dma_start(xlet[:], xle[:])
        xlt3 = xlt[:].rearrange("p (t e) -> p t e", e=E)

        gm1 = rpool.tile([128, TM], f32, tag="gm1")
        nc.vector.reduce_max(gm1[:], xlt3, axis=AX.X)
        gtmp = rpool.tile([128, TM * E], f32, tag="gtmp")
        nc.vector.tensor_tensor(
            gtmp[:].rearrange("p (t e) -> p t e", e=E),
            xlt3, gm1[:].broadcast_to([128, TM, E]), op=OP.is_equal)
        nc.vector.tensor_scalar(gtmp[:], gtmp[:], NEG, None, op0=OP.mult)
        nc.vector.tensor_add(gtmp[:], gtmp[:], xlt[:])
        gm2 = rpool.tile([128, TM], f32, tag="gm2")
        nc.vector.reduce_max(gm2[:], gtmp[:].rearrange("p (t e) -> p t e", e=E), axis=AX.X)

        gden = rpool.tile([128, TM], f32, tag="gden")
        nc.vector.tensor_sub(gden[:], gm2[:], gm1[:])
        nc.scalar.activation(gden[:], gden[:], AF.Exp)
        nc.vector.tensor_scalar(gden[:], gden[:], 1.0, None, op0=OP.add)
        nc.vector.reciprocal(gden[:], gden[:])
        gg = rpool.tile([128, TM], f32, tag="gg")
        nc.vector.tensor_sub(gg[:], xlet[:], gm1[:])
        nc.scalar.activation(gg[:], gg[:], AF.Exp)
        nc.vector.tensor_mul(gg[:], gg[:], gden[:])


        # alternate issue engines: SP and ACT both issue HWDGE DMAs, halving
        # the per-instruction issue serialization on the critical startup path
        for k in range(8):
            load_w(k, nc.sync)
            load_x(0, k, nc.sync)
        for c in range(1, len(XCH)):
            for k in range(8):
                load_x(c, k, nc.sync)

        # ---- expert compute: y[t, :] = g[t] * (x[t, :] @ W) ----
        for m in range(TM):
            ys = ypool.tile([128, H], bf16, tag="ys")
            for n in range(2):
                acc = psum.tile([128, 512], f32, tag="acc")
                c, co = M2C[m]
                for k in range(8):
                    nc.tensor.matmul(
                        acc[:],
                        xk[k][c][:, co:co + 128],
                        wk[k][:, n * 512:(n + 1) * 512],
                        start=(k == 0), stop=(k == 7),
                    )
                nc.scalar.activation(
                    ys[:, n * 512:(n + 1) * 512], acc[:], AF.Copy,
                    scale=gg[:, m:m + 1])
            nc.scalar.dma_start(y[m * 128:(m + 1) * 128, :], ys[:])

        # ---- gates output for this core's token shard ----
        lgt = rpool.tile([128, TSH * E], f32, tag="lgt")
        nc.gpsimd.dma_start(lgt[:], lg[:])
        lgt3 = lgt[:].rearrange("p (t e) -> p t e", e=E)

        m1 = rpool.tile([128, TSH], f32, tag="m1")
        nc.vector.reduce_max(m1[:], lgt3, axis=AX.X)
        eq1 = rpool.tile([128, TSH * E], f32, tag="eq1")
        nc.vector.tensor_tensor(
            eq1[:].rearrange("p (t e) -> p t e", e=E),
            lgt3, m1[:].broadcast_to([128, TSH, E]), op=OP.is_equal)
        nc.vector.tensor_scalar(eq1[:], eq1[:], NEG, None, op0=OP.mult)
        nc.vector.tensor_add(eq1[:], eq1[:], lgt[:])  # eq1 now = lg with top-1 masked
        m2 = rpool.tile([128, TSH], f32, tag="m2")
        nc.vector.reduce_max(m2[:], eq1[:].rearrange("p (t e) -> p t e", e=E), axis=AX.X)
        eq2 = rpool.tile([128, TSH * E], f32, tag="eq2")
        nc.vector.tensor_tensor(
            eq2[:].rearrange("p (t e) -> p t e", e=E),
            eq1[:].rearrange("p (t e) -> p t e", e=E),
            m2[:].broadcast_to([128, TSH, E]), op=OP.is_equal)
        nc.vector.tensor_scalar(eq2[:], eq2[:], NEG, None, op0=OP.mult)
        nc.vector.tensor_add(eq2[:], eq2[:], eq1[:])  # = lg with top-2 masked
        m3 = rpool.tile([128, TSH], f32, tag="m3")
        nc.vector.reduce_max(m3[:], eq2[:].rearrange("p (t e) -> p t e", e=E), axis=AX.X)

        # gates = (lg >= m2) * exp(lg - m1) / (1 + exp(m2 - m1))
        den = rpool.tile([128, TSH], f32, tag="den")
        nc.vector.tensor_sub(den[:], m2[:], m1[:])
        nc.scalar.activation(den[:], den[:], AF.Exp)
        nc.vector.tensor_scalar(den[:], den[:], 1.0, None, op0=OP.add)
        nc.vector.reciprocal(den[:], den[:])

        gates = rpool.tile([128, TSH * E], f32, tag="gates")
        nc.vector.tensor_tensor(
            gates[:].rearrange("p (t e) -> p t e", e=E),
            lgt3, m1[:].broadcast_to([128, TSH, E]), op=OP.subtract)
        nc.scalar.activation(gates[:], gates[:], AF.Exp)
        msk = rpool.tile([128, TSH * E], f32, tag="msk")
        nc.vector.tensor_tensor(
            msk[:].rearrange("p (t e) -> p t e", e=E),
            lgt3, m2[:].broadcast_to([128, TSH, E]), op=OP.is_ge)
        nc.vector.tensor_mul(gates[:], gates[:], msk[:])
        nc.vector.tensor_tensor(
            gates[:].rearrange("p (t e) -> p t e", e=E),
            gates[:].rearrange("p (t e) -> p t e", e=E),
            den[:].broadcast_to([128, TSH, E]), op=OP.mult)
        nc.scalar.dma_start(gsh[:], gates[:])

        # ---- load partial: sum_t where(lg > m3, cdf((cl-m3)/ns), cdf((cl-m2)/ns)) ----
        clt = rpool.tile([128, TSH * E], f32, tag="clt")
        nc.gpsimd.dma_start(clt[:], cl[:])
        nst = rpool.tile([128, TSH * E], f32, tag="nst")
        nc.gpsimd.dma_start(nst[:], nsd[:])
        clt3 = clt[:].rearrange("p (t e) -> p t e", e=E)
        nc.vector.reciprocal(nst[:], nst[:])

        def norm_cdf(out_t, thr):
            # out = 0.5 * erf((cl - thr)/ns / sqrt(2)) + 0.5
            nc.vector.tensor_tensor(
                out_t[:].rearrange("p (t e) -> p t e", e=E),
                clt3, thr[:].broadcast_to([128, TSH, E]), op=OP.subtract)
            nc.vector.tensor_tensor(out_t[:], out_t[:], nst[:], op=OP.mult)
            # clamp to the Erf LUT's safe range; erf(+-5.66) == 1.0 in f32
            nc.vector.tensor_scalar(out_t[:], out_t[:], 8.0, -8.0,
                                    op0=OP.min, op1=OP.max)
            nc.scalar.activation(out_t[:], out_t[:], AF.Erf,
                                 scale=float(1.0 / np.sqrt(2.0)))
            nc.vector.tensor_scalar(out_t[:], out_t[:], 0.5, 0.5,
                                    op0=OP.mult, op1=OP.add)

        pin = rpool.tile([128, TSH * E], f32, tag="pin")
        norm_cdf(pin, m3)
        pout = rpool.tile([128, TSH * E], f32, tag="pout")
        norm_cdf(pout, m2)
        sel = rpool.tile([128, TSH * E], f32, tag="sel")
        nc.vector.tensor_tensor(
            sel[:].rearrange("p (t e) -> p t e", e=E),
            lgt3, m3[:].broadcast_to([128, TSH, E]), op=OP.is_gt)
        nc.vector.tensor_sub(pin[:], pin[:], pout[:])
        nc.vector.tensor_mul(pin[:], pin[:], sel[:])
        nc.vector.tensor_add(pin[:], pin[:], pout[:])

        lpart = rpool.tile([128, E], f32, tag="lpart")
        nc.vector.reduce_sum(lpart[:], pin[:].rearrange("p (t e) -> p e t", e=E),
                             axis=AX.X)
        ones = rpool.tile([128, 1], f32, tag="ones")
        nc.vector.memset(ones[:], 1.0)
        lacc = pl.tile([1, E], f32, tag="lacc")
        nc.tensor.matmul(lacc[:], ones[:], lpart[:], start=True, stop=True)
        lout = rpool.tile([1, E], f32, tag="lout")
        nc.vector.tensor_copy(lout[:], lacc[:])
        nc.scalar.dma_start(lsh[:], lout[:])

    nc.compile()
    return nc


def _get_nc():
    if "nc" not in _compiled:
        _compiled["nc"] = _build_nc()
    return _compiled["nc"]


def _route(logits):
    """Host-side top-2 routing -> per-expert token index lists (the all-to-all)."""
    order = np.argsort(-logits, axis=1, kind="stable")
    top2 = order[:, :TOPK]
    sels, cnts = [], []
    for e in range(E):
        sel = np.nonzero((top2[:, 0] == e) | (top2[:, 1] == e))[0]
        assert len(sel) <= C, f"expert {e} overflow: {len(sel)} > {C}"
        sels.append(sel)
        cnts.append(len(sel))
    return sels, cnts


def _make_in_maps(net, logits, clean_logits, noise_std, W, sels, cnts):
    def pack(a, t):
        # [t*128, F] -> [128, t*F] matching the on-chip "p (t f)" layout
        return np.ascontiguousarray(
            a.reshape(t, 128, -1).transpose(1, 0, 2).reshape(128, -1))

    in_maps = []
    for e in range(E):
        sel = sels[e]
        idx = np.zeros(C, dtype=np.int64)
        idx[:len(sel)] = sel
        xg = net[idx]
        xg[len(sel):] = 0.0
        xgT = np.ascontiguousarray(xg.T).astype(ml_dtypes.bfloat16)
        xlrows = logits[idx].copy()
        xlrows[len(sel):] = 0.0
        s = slice(e * NS, (e + 1) * NS)
        in_maps.append({
            "xgt": xgT,
            "wq": W[e].astype(ml_dtypes.bfloat16),
            "xl": pack(xlrows, TM),
            "xle": pack(xlrows[:, e:e + 1], TM),
            "lg": pack(logits[s], TSH),
            "cl": pack(clean_logits[s], TSH),
            "nsd": pack(noise_std[s], TSH),
        })
    return in_maps


def kernel(net, logits, clean_logits, noise_std, W):
    global LAST_RESULTS
    from concourse.bass_utils import run_bass_kernel_spmd

    net = np.asarray(net, dtype=np.float32)
    logits = np.asarray(logits, dtype=np.float32)
    clean_logits = np.asarray(clean_logits, dtype=np.float32)
    noise_std = np.asarray(noise_std, dtype=np.float32)
    W = np.asarray(W, dtype=np.float32)

    sels, cnts = _route(logits)
    in_maps = _make_in_maps(net, logits, clean_logits, noise_std, W, sels, cnts)

    nc = _get_nc()
    res = run_bass_kernel_spmd(
        nc, in_maps, core_ids=list(range(NCORE)), trace=TRACE)
    LAST_RESULTS = res

    out = np.zeros((N, H), dtype=np.float32)
    gates = np.empty((N, E), dtype=np.float32)
    load = np.zeros(E, dtype=np.float32)
    for e in range(E):
        r = res.results[e]
        out[sels[e]] += r["y"][:cnts[e]].astype(np.float32)
        gates[e * NS:(e + 1) * NS] = (
            r["gsh"].reshape(128, TSH, E).transpose(1, 0, 2).reshape(NS, E))
        load += r["lsh"].reshape(E)
    return out, gates, load


# revision 41
# speedup vs baseline: 1.0632x; 1.0066x over previous
"""Trainium2 Bass kernel for nn_MoEDispatcher (noisy top-k MoE routing + dispatch).

Strategy (expert-parallel, per sharding hint):
  - Host computes top-2 routing from logits and "all-to-all" routes tokens:
    for each expert e, gathers the net rows of tokens whose top-2 contains e
    into a capacity-padded buffer (pre-transposed to [D, C] so the device
    needs no on-chip transposes), plus the router rows for those tokens.
  - Core e computes:
      * softmax-of-top-2 gate for each gathered token (from its logits row)
      * y = gate * (x @ W[e])  via bf16 matmuls, f32 accumulate
      * the dense gates output for its 1/8 token shard
      * the noisy-load partial for its 1/8 token shard
  - Host scatter-adds y rows back per token (the return all-to-all + combine),
    concatenates gate shards, sums load partials.

All model math (gates, load, matmul, gate-weighting) runs on device; the host
does routing-index construction, data movement, and layout only.
"""

import os
import numpy as np
import ml_dtypes

N, E, TOPK, D, H = 8192, 8, 2, 1024, 1024
NCORE = 8
NS = N // NCORE          # tokens per shard for gates/load outputs
C = 2304                 # gather capacity per expert (multiple of 128)
TM = C // 128            # token tiles per core
TSH = NS // 128          # shard token groups
NEG = -1.0e30

TRACE = False            # set True (e.g. from test.py) to neuron-profile
LAST_RESULTS = None      # BassKernelResults of the last kernel() call

_compiled = {}


def _build_nc():
    from contextlib import ExitStack
    import concourse.bass as bass
    import concourse.tile as tile
    from concourse import bacc, mybir

    f32 = mybir.dt.float32
    bf16 = mybir.dt.bfloat16
    AX = mybir.AxisListType
    OP = mybir.AluOpType
    AF = mybir.ActivationFunctionType

    nc = bacc.Bacc("TRN2", target_bir_lowering=False, debug=False)

    # router tensors arrive host-packed as [128, F] so every DMA is 128 dense rows
    xgT = nc.dram_tensor("xgt", [D, C], bf16, kind="ExternalInput").ap()
    wq = nc.dram_tensor("wq", [D, H], bf16, kind="ExternalInput").ap()
    xl = nc.dram_tensor("xl", [128, TM * E], f32, kind="ExternalInput").ap()
    xle = nc.dram_tensor("xle", [128, TM], f32, kind="ExternalInput").ap()
    lg = nc.dram_tensor("lg", [128, TSH * E], f32, kind="ExternalInput").ap()
    cl = nc.dram_tensor("cl", [128, TSH * E], f32, kind="ExternalInput").ap()
    nsd = nc.dram_tensor("nsd", [128, TSH * E], f32, kind="ExternalInput").ap()

    y = nc.dram_tensor("y", [C, H], bf16, kind="ExternalOutput").ap()
    gsh = nc.dram_tensor("gsh", [128, TSH * E], f32, kind="ExternalOutput").ap()
    lsh = nc.dram_tensor("lsh", [1, E], f32, kind="ExternalOutput").ap()

    with tile.TileContext(nc) as tc, ExitStack() as ctx:
        wpool = ctx.enter_context(tc.tile_pool(name="w", bufs=1))
        xpool = ctx.enter_context(tc.tile_pool(name="x", bufs=1))
        rpool = ctx.enter_context(tc.tile_pool(name="r", bufs=1))
        ypool = ctx.enter_context(tc.tile_pool(name="y", bufs=4))
        psum = ctx.enter_context(tc.tile_pool(name="ps", bufs=6, space="PSUM"))
        pl = ctx.enter_context(tc.tile_pool(name="pl", bufs=1, space="PSUM"))

        # ---- stream in W[e] and gathered-token xT tiles (bf16) ----
        # big DMA rows (DMA is ~42ns/descriptor-run); small first chunk and
        # first-needed-first emission so the first matmuls start early
        XCH = [512, 1792]              # column-chunk widths, sum = C
        XOFF = [0, 512]
        M2C = []                       # m-tile -> (chunk, offset-within-chunk)
        for m in range(TM):
            col = m * 128
            c = max(i for i in range(len(XOFF)) if XOFF[i] <= col)
            M2C.append((c, col - XOFF[c]))
        wk = [None] * 8
        xk = [[None] * len(XCH) for _ in range(8)]

        def load_w(k, eng):
            wt = wpool.tile([128, H], bf16, tag=f"w{k}")
            eng.dma_start(wt[:], wq[k * 128:(k + 1) * 128, :])
            wk[k] = wt

        def load_x(c, k, eng):
            c0 = XOFF[c]
            xt = xpool.tile([128, XCH[c]], bf16, tag=f"x{k}c{c}")
            eng.dma_start(xt[:], xgT[k * 128:(k + 1) * 128, c0:c0 + XCH[c]])
            xk[k][c] = xt

        # ---- gate for each gathered token: g = exp(le-m1) / (1+exp(m2-m1)) ----
        xlt = rpool.tile([128, TM * E], f32, tag="xlt")
        nc.gpsimd.dma_start(xlt[:], xl[:])
        xlet = rpool.tile([128, TM], f32, tag="xlet")
        nc.gpsimd.dma_start(xlet[:], xle[:])
        xlt3 = xlt[:].rearrange("p (t e) -> p t e", e=E)

        gm1 = rpool.tile([128, TM], f32, tag="gm1")
        nc.vector.reduce_max(gm1[:], xlt3, axis=AX.X)
        gtmp = rpool.tile([128, TM * E], f32, tag="gtmp")
        nc.vector.tensor_tensor(
            gtmp[:].rearrange("p (t e) -> p t e", e=E),
            xlt3, gm1[:].broadcast_to([128, TM, E]), op=OP.is_equal)
        nc.vector.tensor_scalar(gtmp[:], gtmp[:], NEG, None, op0=OP.mult)
        nc.vector.tensor_add(gtmp[:], gtmp[:], xlt[:])
        gm2 = rpool.tile([128, TM], f32, tag="gm2")
        nc.vector.reduce_max(gm2[:], gtmp[:].rearrange("p (t e) -> p t e", e=E), axis=AX.X)

        gden = rpool.tile([128, TM], f32, tag="gden")
        nc.vector.tensor_sub(gden[:], gm2[:], gm1[:])
        nc.scalar.activation(gden[:], gden[:], AF.Exp)
        nc.vector.tensor_scalar(gden[:], gden[:], 1.0, None, op0=OP.add)
        nc.vector.reciprocal(gden[:], gden[:])
        gg = rpool.tile([128, TM], f32, tag="gg")
        nc.vector.tensor_sub(gg[:], xlet[:], gm1[:])
        nc.scalar.activation(gg[:], gg[:], AF.Exp)
        nc.vector.tensor_mul(gg[:], gg[:], gden[:])


        # k-interleaved so the first matmul group's tiles land first
        for k in range(8):
            load_w(k, nc.sync)
            load_x(0, k, nc.sync)
        for c in range(1, len(XCH)):
            for k in range(8):
                load_x(c, k, nc.sync)

        # ---- expert compute: y[t, :] = g[t] * (x[t, :] @ W) ----
        for m in range(TM):
            ys = ypool.tile([128, H], bf16, tag="ys")
            for n in range(2):
                acc = psum.tile([128, 512], f32, tag="acc")
                c, co = M2C[m]
                for k in range(8):
                    nc.tensor.matmul(
                        acc[:],
                        xk[k][c][:, co:co + 128],
                        wk[k][:, n * 512:(n + 1) * 512],
                        start=(k == 0), stop=(k == 7),
                    )
                nc.scalar.activation(
                    ys[:, n * 512:(n + 1) * 512], acc[:], AF.Copy,
                    scale=gg[:, m:m + 1])
            nc.scalar.dma_start(y[m * 128:(m + 1) * 128, :], ys[:])

        # ---- gates output for this core's token shard ----
        lgt = rpool.tile([128, TSH * E], f32, tag="lgt")
        nc.gpsimd.dma_start(lgt[:], lg[:])
        lgt3 = lgt[:].rearrange("p (t e) -> p t e", e=E)

        m1 = rpool.tile([128, TSH], f32, tag="m1")
        nc.vector.reduce_max(m1[:], lgt3, axis=AX.X)
        eq1 = rpool.tile([128, TSH * E], f32, tag="eq1")
        nc.vector.tensor_tensor(
            eq1[:].rearrange("p (t e) -> p t e", e=E),
            lgt3, m1[:].broadcast_to([128, TSH, E]), op=OP.is_equal)
        nc.vector.tensor_scalar(eq1[:], eq1[:], NEG, None, op0=OP.mult)
        nc.vector.tensor_add(eq1[:], eq1[:], lgt[:])  # eq1 now = lg with top-1 masked
        m2 = rpool.tile([128, TSH], f32, tag="m2")
        nc.vector.reduce_max(m2[:], eq1[:].rearrange("p (t e) -> p t e", e=E), axis=AX.X)
        eq2 = rpool.tile([128, TSH * E], f32, tag="eq2")
        nc.vector.tensor_tensor(
            eq2[:].rearrange("p (t e) -> p t e", e=E),
            eq1[:].rearrange("p (t e) -> p t e", e=E),
            m2[:].broadcast_to([128, TSH, E]), op=OP.is_equal)
        nc.vector.tensor_scalar(eq2[:], eq2[:], NEG, None, op0=OP.mult)
        nc.vector.tensor_add(eq2[:], eq2[:], eq1[:])  # = lg with top-2 masked
        m3 = rpool.tile([128, TSH], f32, tag="m3")
        nc.vector.reduce_max(m3[:], eq2[:].rearrange("p (t e) -> p t e", e=E), axis=AX.X)

        # gates = (lg >= m2) * exp(lg - m1) / (1 + exp(m2 - m1))
        den = rpool.tile([128, TSH], f32, tag="den")
        nc.vector.tensor_sub(den[:], m2[:], m1[:])
        nc.scalar.activation(den[:], den[:], AF.Exp)
        nc.vector.tensor_scalar(den[:], den[:], 1.0, None, op0=OP.add)
        nc.vector.reciprocal(den[:], den[:])

        gates = rpool.tile([128, TSH * E], f32, tag="gates")
        nc.vector.tensor_tensor(
            gates[:].rearrange("p (t e) -> p t e", e=E),
            lgt3, m1[:].broadcast_to([128, TSH, E]), op=OP.subtract)
        nc.scalar.activation(gates[:], gates[:], AF.Exp)
        msk = rpool.tile([128, TSH * E], f32, tag="msk")
        nc.vector.tensor_tensor(
            msk[:].rearrange("p (t e) -> p t e", e=E),
            lgt3, m2[:].broadcast_to([128, TSH, E]), op=OP.is_ge)
        nc.vector.tensor_mul(gates[:], gates[:], msk[:])
        nc.vector.tensor_tensor(
            gates[:].rearrange("p (t e) -> p t e", e=E),
            gates[:].rearrange("p (t e) -> p t e", e=E),
            den[:].broadcast_to([128, TSH, E]), op=OP.mult)
        nc.scalar.dma_start(gsh[:], gates[:])

        # ---- load partial: sum_t where(lg > m3, cdf((cl-m3)/ns), cdf((cl-m2)/ns)) ----
        clt = rpool.tile([128, TSH * E], f32, tag="clt")
        nc.gpsimd.dma_start(clt[:], cl[:])
        nst = rpool.tile([128, TSH * E], f32, tag="nst")
        nc.gpsimd.dma_start(nst[:], nsd[:])
        clt3 = clt[:].rearrange("p (t e) -> p t e", e=E)
        nc.vector.reciprocal(nst[:], nst[:])

        def norm_cdf(out_t, thr):
            # out = 0.5 * erf((cl - thr)/ns / sqrt(2)) + 0.5
            nc.vector.tensor_tensor(
                out_t[:].rearrange("p (t e) -> p t e", e=E),
                clt3, thr[:].broadcast_to([128, TSH, E]), op=OP.subtract)
            nc.vector.tensor_tensor(out_t[:], out_t[:], nst[:], op=OP.mult)
            # clamp to the Erf LUT's safe range; erf(+-5.66) == 1.0 in f32
            nc.vector.tensor_scalar(out_t[:], out_t[:], 8.0, -8.0,
                                    op0=OP.min, op1=OP.max)
            nc.scalar.activation(out_t[:], out_t[:], AF.Erf,
                                 scale=float(1.0 / np.sqrt(2.0)))
            nc.vector.tensor_scalar(out_t[:], out_t[:], 0.5, 0.5,
                                    op0=OP.mult, op1=OP.add)

        pin = rpool.tile([128, TSH * E], f32, tag="pin")
        norm_cdf(pin, m3)
        pout = rpool.tile([128, TSH * E], f32, tag="pout")
        norm_cdf(pout, m2)
        sel = rpool.tile([128, TSH * E], f32, tag="sel")
        nc.vector.tensor_tensor(
            sel[:].rearrange("p (t e) -> p t e", e=E),
            lgt3, m3[:].broadcast_to([128, TSH, E]), op=OP.is_gt)
        nc.vector.tensor_sub(pin[:], pin[:], pout[:])
        nc.vector.tensor_mul(pin[:], pin[:], sel[:])
        nc.vector.tensor_add(pin[:], pin[:], pout[:])

        lpart = rpool.tile([128, E], f32, tag="lpart")
        nc.vector.reduce_sum(lpart[:], pin[:].rearrange("p (t e) -> p e t", e=E),
                             axis=AX.X)
        ones = rpool.tile([128, 1], f32, tag="ones")
        nc.vector.memset(ones[:], 1.0)
        lacc = pl.tile([1, E], f32, tag="lacc")
        nc.tensor.matmul(lacc[:], ones[:], lpart[:], start=True, stop=True)
        lout = rpool.tile([1, E], f32, tag="lout")
        nc.vector.tensor_copy(lout[:], lacc[:])
        nc.scalar.dma_start(lsh[:], lout[:])

    nc.compile()
    return nc


def _get_nc():
    if "nc" not in _compiled:
        _compiled["nc"] = _build_nc()
    return _compiled["nc"]


def _route(logits):
    """Host-side top-2 routing -> per-expert token index lists (the all-to-all)."""
    order = np.argsort(-logits, axis=1, kind="stable")
    top2 = order[:, :TOPK]
    sels, cnts = [], []
    for e in range(E):
        sel = np.nonzero((top2[:, 0] == e) | (top2[:, 1] == e))[0]
        assert len(sel) <= C, f"expert {e} overflow: {len(sel)} > {C}"
        sels.append(sel)
        cnts.append(len(sel))
    return sels, cnts


def _make_in_maps(net, logits, clean_logits, noise_std, W, sels, cnts):
    def pack(a, t):
        # [t*128, F] -> [128, t*F] matching the on-chip "p (t f)" layout
        return np.ascontiguousarray(
            a.reshape(t, 128, -1).transpose(1, 0, 2).reshape(128, -1))

    in_maps = []
    for e in range(E):
        sel = sels[e]
        idx = np.zeros(C, dtype=np.int64)
        idx[:len(sel)] = sel
        xg = net[idx]
        xg[len(sel):] = 0.0
        xgT = np.ascontiguousarray(xg.T).astype(ml_dtypes.bfloat16)
        xlrows = logits[idx].copy()
        xlrows[len(sel):] = 0.0
        s = slice(e * NS, (e + 1) * NS)
        in_maps.append({
            "xgt": xgT,
            "wq": W[e].astype(ml_dtypes.bfloat16),
            "xl": pack(xlrows, TM),
            "xle": pack(xlrows[:, e:e + 1], TM),
            "lg": pack(logits[s], TSH),
            "cl": pack(clean_logits[s], TSH),
            "nsd": pack(noise_std[s], TSH),
        })
    return in_maps


def kernel(net, logits, clean_logits, noise_std, W):
    global LAST_RESULTS
    from concourse.bass_utils import run_bass_kernel_spmd

    net = np.asarray(net, dtype=np.float32)
    logits = np.asarray(logits, dtype=np.float32)
    clean_logits = np.asarray(clean_logits, dtype=np.float32)
    noise_std = np.asarray(noise_std, dtype=np.float32)
    W = np.asarray(W, dtype=np.float32)

    sels, cnts = _route(logits)
    in_maps = _make_in_maps(net, logits, clean_logits, noise_std, W, sels, cnts)

    nc = _get_nc()
    res = run_bass_kernel_spmd(
        nc, in_maps, core_ids=list(range(NCORE)), trace=TRACE)
    LAST_RESULTS = res

    out = np.zeros((N, H), dtype=np.float32)
    gates = np.empty((N, E), dtype=np.float32)
    load = np.zeros(E, dtype=np.float32)
    for e in range(E):
        r = res.results[e]
        out[sels[e]] += r["y"][:cnts[e]].astype(np.float32)
        gates[e * NS:(e + 1) * NS] = (
            r["gsh"].reshape(128, TSH, E).transpose(1, 0, 2).reshape(NS, E))
        load += r["lsh"].reshape(E)
    return out, gates, load
